# revision 1
# baseline (speedup 1.0000x reference)
"""Trainium2 Bass kernel for nn_DaleDendriticMLP (topk_masking).

Strategy: tensor-parallel over the 2048 hidden units across 8 NeuronCores
(256 units per core). All value-bearing arithmetic is fp32: the k-winners
and abs-argmax boundary decisions amplify any value noise into >2e-2
output error (one swapped winner costs ~2.6%), so matmul inputs cannot be
quantized. Only the 0/1 masks are stored as fp8 (exact multiply, halves
mask DMA traffic).

Per layer, each core computes its 256-unit shard's masked feedforward +
dendritic gating, extracts its local top-32 gated values AND their unit
indices per sample (the global top-102 per shard is <=32 with ~5.8 sigma
margin), and AllGathers only these sparse candidate lists (49KB/core
instead of a 295KB dense activation gather). Every core then finds the
exact per-row 102nd-largest candidate (13 rounds of max8/match_replace
over the merged 256), reconstructs the dense masked hidden vector by
gpsimd local_scatter of the candidate values (fp32 scattered as u16
lo/hi pairs into a bitcast row; sub-threshold entries are zeroed by a
fused compare-multiply afterwards), and PE-transposes it into the
[unit, batch] layout the next layer's matmuls need. The Dale (EiDense)
output head is computed redundantly on every core.

Engine placement: PE matmuls/transposes; DVE mask-multiplies (in-place
into the streamed weight tiles), segment max/min reduces, top-k, gating
select, threshold mask; ACT sigmoid and PSUM->SBUF copies; GPSIMD
scatters + collectives. Weight streams issue on the SP DMA queue; the
payload/collective/readback chain issues on the GPSIMD queue so neither
blocks the other behind a semaphore wait (head-of-line).

Host side does layout-only work: sharding, transposes, dtype casts.
"""

import os

os.environ.pop("JAX_PLATFORMS", None)
if not os.environ.get("BASS_TRACE"):
    os.environ["BASS_NEVER_TRACE"] = "1"

import numpy as np

import concourse.bacc as bacc
import concourse.tile as tile
import concourse.mybir as mybir
from concourse.bass_utils import run_bass_kernel_spmd

R = 8                    # cores
B = 256                  # batch
HID = 2048
U = HID // R             # 256 units per core
D_IN = 2048
D_CTX = 1024
KI = D_IN // 128         # 16 input K-chunks
KC = D_CTX // 128        # 8 context K-chunks
KH = HID // 128          # 16 hidden K-chunks
NSEG = 10
OUT = 100
KWIN = 102
LOC_ROUNDS = 4           # local top-32 per core
MERGE_ROUNDS = 13        # top-104 of merged 256
NEG = -1.0e30

f32 = mybir.dt.float32
fp8 = mybir.dt.float8e4
u16 = mybir.dt.uint16
i16 = mybir.dt.int16
X = mybir.AxisListType.X
ALU = mybir.AluOpType
AF = mybir.ActivationFunctionType

_CACHE = {}
LAST_RESULT = None
ABL = set(x for x in os.environ.get("ABL", "").split(",") if x)
DBG = os.environ.get("DBG", "0") == "1"

VAL_N = 8 * LOC_ROUNDS               # 32 candidates per row per core
# payload: top-32 values (f32) + their local unit indices (u16, bitcast
# into f32 slots). The winners are reconstructed from these by scatter.
PAY = 2 * 128 * VAL_N + 128 * VAL_N  # 8192 vals + 4096 idx-slots = 12288


def _build(n_iters: int = 1):
    nc = bacc.Bacc(
        "TRN2",
        target_bir_lowering=False,
        debug=False,
        enable_asserts=False,
        num_devices=R,
    )

    dram = {}

    def din(name, shape, dt=f32):
        dram[name] = nc.dram_tensor(name, shape, dt, kind="ExternalInput")
        return dram[name]

    din("xT", [D_IN, B])
    din("cT", [D_CTX, B])
    for L in (1, 2):
        din(f"wT{L}", [D_IN if L == 1 else HID, U])
        din(f"mwT{L}", [D_IN if L == 1 else HID, U], fp8)
        din(f"sgT{L}", [D_CTX, 2, NSEG, 128])
        din(f"msT{L}", [D_CTX, 2, NSEG, 128], fp8)
        din(f"b{L}", [1, U])
    din("wexT", [HID, OUT])
    din("wixT", [HID, 1])
    din("weiT", [1, OUT])
    din("bout", [1, OUT])
    out_d = nc.dram_tensor("out", [B, OUT], f32, kind="ExternalOutput")

    ident_d = nc.inline_tensor(np.eye(128, dtype=np.float32), "ident")
    ones_d = nc.inline_tensor(np.ones((1, 128), np.float32), "ones_row")
    # expanded-u16-scatter offsets: candidate block r (32 cands each) lands in
    # quarter r//2 at u16 base 512*(r%2); even/odd slots hold lo/hi halves
    j = np.arange(R * VAL_N)
    roffE_np = np.broadcast_to(
        (512 * ((j // VAL_N) % 2)).astype(np.uint16), (128, R * VAL_N))
    roffE_d = nc.inline_tensor(np.ascontiguousarray(roffE_np), "roffE")
    roffO_d = nc.inline_tensor(np.ascontiguousarray(roffE_np + 1), "roffO")

    # One fused AllGather per layer: payload = yT shard + top-32 lists
    gath_g = {
        (L, i): nc.dram_tensor(f"gath_g{L}_{i}", [R * PAY], f32,
                               kind="Internal", addr_space="Shared")
        for L in (1, 2) for i in range(n_iters)
    }
    groups = [list(range(R))]

    with tile.TileContext(nc) as tc:
        with (
            tc.tile_pool(name="pa", bufs=1) as pa,          # persistent SBUF
            tc.tile_pool(name="pin", bufs=1) as pin,        # layer input
            tc.tile_pool(name="pw", bufs=2) as pw,          # masked W
            tc.tile_pool(name="pmw", bufs=1) as pmw,        # W-mask (fp8)
            tc.tile_pool(name="psg", bufs=8) as psg,        # seg weights (f32)
            tc.tile_pool(name="pms", bufs=4) as pms,        # seg masks (fp8)
            tc.tile_pool(name="pdram", bufs=1, space="DRAM") as pdram,
            tc.tile_pool(name="pp_y", bufs=1, space="PSUM") as pp_y,
            tc.tile_pool(name="pp_d", bufs=1, space="PSUM") as pp_d,
            tc.tile_pool(name="pp_m", bufs=1, space="PSUM") as pp_m,
        ):
            from concourse import library_config

            nc.gpsimd.load_library(library_config.local_scatter)

            ident = pa.tile([128, 128], f32, tag="ident")
            nc.sync.dma_start(ident[:], ident_d[:])
            ones = pa.tile([1, 128], f32, tag="ones")
            nc.sync.dma_start(ones[:], ones_d[:])
            roffE = pa.tile([128, R * VAL_N], u16, tag="roffE")
            nc.sync.dma_start(roffE[:], roffE_d[:])
            roffO = pa.tile([128, R * VAL_N], u16, tag="roffO")
            nc.sync.dma_start(roffO[:], roffO_d[:])

            cT = pa.tile([128, KC, B], f32, tag="cT")
            nc.sync.dma_start(cT[:], dram["cT"][:].rearrange("(k p) b -> p k b", p=128))

            def emit_layer(L, it, in_sb, nk):
                """in_sb: [128, nk, B] f32 transposed input; returns next input."""
                wT_d, mwT_d = dram[f"wT{L}"], dram[f"mwT{L}"]
                sgT_d, msT_d = dram[f"sgT{L}"], dram[f"msT{L}"]

                def seg_group(uh, g2):
                    # L1's first slice issues on the (empty) GPSIMD queue so
                    # it jumps ahead of the SP weight stream in DMA arrival
                    # order -- the first dendrite matmul starts ~8us earlier
                    eng = nc.gpsimd if (L == 1 and uh == 0 and g2 == 0) \
                        else nc.sync
                    rows = slice(256 * g2, 256 * (g2 + 1))
                    sg = psg.tile([128, 2, NSEG * 128], f32, tag="sg")
                    eng.dma_start(
                        sg[:].rearrange("p k (s u) -> p k s u", s=NSEG),
                        sgT_d[rows, uh].rearrange("(k p) s u -> p k s u",
                                                  p=128))
                    ms = pms.tile([128, 2, NSEG * 128], fp8, tag="ms")
                    eng.dma_start(
                        ms[:].rearrange("p k (s u) -> p k s u", s=NSEG),
                        msT_d[rows, uh].rearrange("(k p) s u -> p k s u",
                                                  p=128))
                    nc.vector.tensor_tensor(sg[:], sg[:], ms[:], op=ALU.mult)
                    return sg

                # --- A: load + mask feedforward weights (in-place, fp8 mask)
                wm = pw.tile([128, nk, U], f32, tag="wm")
                nc.sync.dma_start(wm[:], wT_d[:].rearrange("(k p) u -> p k u", p=128))
                mwc = pmw.tile([128, nk, U], fp8, tag="mwc")
                nc.sync.dma_start(mwc[:], mwT_d[:].rearrange("(k p) u -> p k u", p=128))
                for h in range(2):
                    sl = slice(h * nk // 2, (h + 1) * nk // 2)
                    nc.vector.tensor_tensor(
                        wm[:, sl, :], wm[:, sl, :], mwc[:, sl, :], op=ALU.mult)

                b_sb = pa.tile([1, U], f32, tag=f"bias{L}")
                nc.sync.dma_start(b_sb[:], dram[f"b{L}"][:])

                # --- C: dendrites d[b,u,s]; running max/min over s ---
                maxd = pa.tile([128, 2 * U], f32, tag="maxd")
                mind = pa.tile([128, 2 * U], f32, tag="mind")
                if "nodend" in ABL:
                    nc.vector.memset(maxd[:], 1.0)
                    nc.vector.memset(mind[:], -0.5)
                for uh in range(2 if "nodend" not in ABL else 0):
                    # stream masked seg weights in four 2-chunk slices,
                    # mask-multiplied in place; matmuls read the slices
                    sgh = [seg_group(uh, g2) for g2 in range(4)]
                    dps = [pp_d.tile([128, NSEG, 128], f32, tag=f"d{bt}",
                                     name=f"d{bt}") for bt in range(2)]
                    dfl = [dps[bt][:].rearrange("p s u -> p (s u)")
                           for bt in range(2)]
                    for g2 in range(4):
                        for k2 in range(2):
                            k = 2 * g2 + k2
                            for bt in range(2):
                                for c0, ncols in ((0, 512), (512, 512),
                                                  (1024, 256)):
                                    nc.tensor.matmul(
                                        dfl[bt][:, c0:c0 + ncols],
                                        lhsT=cT[:, k, 128 * bt:128 * (bt + 1)],
                                        rhs=sgh[g2][:, k2, c0:c0 + ncols],
                                        start=(k == 0), stop=(k == KC - 1))
                    for bt in range(2):
                        v = dps[bt][:].rearrange("p s u -> p u s")
                        col = U * bt + 128 * uh
                        nc.vector.tensor_reduce(
                            maxd[:, col:col + 128], v, axis=X, op=ALU.max)
                        nc.vector.tensor_reduce(
                            mind[:, col:col + 128], v, axis=X, op=ALU.min)

                # --- B: feedforward y = in @ Wm.T + b -> y_all [128,512]
                y_all = pa.tile([128, 2 * U], f32, tag="y_all")
                if "noff" in ABL:
                    nc.vector.memset(y_all[:], 0.25)
                for bt in range(2 if "noff" not in ABL else 0):
                    yp = pp_y.tile([128, U], f32, tag="yp")
                    for k in range(nk):
                        nc.tensor.matmul(
                            yp[:], lhsT=in_sb[:, k, 128 * bt:128 * (bt + 1)],
                            rhs=wm[:, k, :], start=(k == 0), stop=False)
                    nc.tensor.matmul(yp[:], lhsT=ones[:], rhs=b_sb[:],
                                     start=False, stop=True)
                    nc.scalar.copy(y_all[:, U * bt:U * (bt + 1)], yp[:])

                # --- D: abs-argmax gating:
                # chosen = (maxd+mind>=0) ? maxd : mind; sig = sigmoid(chosen)
                g = pa.tile([128, 2 * U], f32, tag="g")
                nc.vector.tensor_tensor(g[:], maxd[:], mind[:], op=ALU.add)
                ga = pa.tile([128, 2 * U], f32, tag="ga")
                nc.vector.scalar_tensor_tensor(
                    ga[:], g[:], 0.0, maxd[:], op0=ALU.is_ge, op1=ALU.mult)
                gb = pa.tile([128, 2 * U], f32, tag="gb")
                nc.vector.scalar_tensor_tensor(
                    gb[:], g[:], 0.0, mind[:], op0=ALU.is_lt, op1=ALU.mult)
                nc.vector.tensor_tensor(ga[:], ga[:], gb[:], op=ALU.add)
                sig = pa.tile([128, 2 * U], f32, tag="sig")
                nc.scalar.activation(sig[:], ga[:], AF.Sigmoid)
                yg = pa.tile([128, 2 * U], f32, tag="yg")
                nc.vector.tensor_tensor(yg[:], y_all[:], sig[:], op=ALU.mult)
                if DBG and L == 1:
                    for nm, t in (("y_all", y_all), ("maxd", maxd),
                                  ("mind", mind), ("sig", sig), ("yg", yg)):
                        dd = nc.dram_tensor(f"dbg_{nm}", [128, 2 * U], f32,
                                            kind="ExternalOutput")
                        nc.sync.dma_start(dd[:], t[:])

                # --- E: local top-k values + indices (destroys yg) ---
                vals_c = pa.tile([128, 2, VAL_N], f32, tag="valsc")
                idx_c = pa.tile([128, 2, VAL_N], u16, tag="idxc")
                for bt in range(2):
                    sc = yg[:, U * bt:U * (bt + 1)]
                    for r in range(LOC_ROUNDS):
                        v8 = vals_c[:, bt, 8 * r:8 * (r + 1)]
                        nc.vector.max(v8, sc)
                        nc.vector.max_index(idx_c[:, bt, 8 * r:8 * (r + 1)],
                                            v8, sc)
                        if r < LOC_ROUNDS - 1:
                            nc.vector.match_replace(sc, v8, sc, NEG)

                NV = 2 * 128 * VAL_N            # 8192 f32 vals
                pay = pdram.tile([PAY], f32, tag="pay")
                nc.gpsimd.dma_start(
                    pay[0:NV].rearrange("(p t j) -> p t j", p=128, t=2),
                    vals_c[:])
                nc.gpsimd.dma_start(
                    pay[NV:].bitcast(u16).rearrange("(p t j) -> p t j",
                                                    p=128, t=2),
                    idx_c[:])
                if "nocc" in ABL:
                    nc.gpsimd.dma_start(gath_g[(L, it)][0:PAY], pay[:])
                else:
                    nc.gpsimd.collective_compute(
                        "AllGather", ALU.bypass, replica_groups=groups,
                        ins=[pay.opt()], outs=[gath_g[(L, it)][:]])
                gath = gath_g[(L, it)]

                # --- G: merge -> exact 102nd-largest threshold; scatter the
                # winning (value, index) pairs back into a dense [b, 2048] row
                hrec = pa.tile([128, 2, HID], f32, tag="hrec")
                for bt in range(2):
                    src_v = gath[:].rearrange(
                        "(r q) -> r q", q=PAY)[:, 0:NV].rearrange(
                        "r (p t j) -> p t r j", p=128, t=2)[:, bt]
                    merged = pa.tile([128, R * VAL_N], f32,
                                     tag=f"mrg{bt}", name=f"mrg{bt}")
                    nc.gpsimd.dma_start(
                        merged[:].rearrange("p (r j) -> p r j", r=R), src_v)
                    cands = pa.tile([128, R * VAL_N], f32,
                                    tag=f"cnd{bt}", name=f"cnd{bt}")
                    nc.vector.tensor_copy(cands[:], merged[:])
                    idxg = pa.tile([128, R * VAL_N], u16,
                                   tag=f"idg{bt}", name=f"idg{bt}")
                    nc.gpsimd.dma_start(
                        idxg[:].rearrange("p (r j) -> p r j", r=R),
                        gath[:].rearrange("(r q) -> r q", q=PAY)[:, NV:]
                        .bitcast(u16).rearrange("r (p t j) -> p t r j",
                                                p=128, t=2)[:, bt])
                    # expanded u16 scatter indices (no threshold dep) and
                    # raw-candidate scatter overlap the merge rounds; the
                    # threshold mask is applied on the dense rows after
                    idxe = pa.tile([128, 2 * R * VAL_N], i16, tag=f"ide{bt}",
                                   name=f"ide{bt}")
                    iev = idxe[:].rearrange("p (j t) -> p t j", t=2)
                    nc.vector.scalar_tensor_tensor(
                        iev[:, 0, :], idxg[:], 2.0, roffE[:],
                        op0=ALU.mult, op1=ALU.add)
                    nc.vector.scalar_tensor_tensor(
                        iev[:, 1, :], idxg[:], 2.0, roffO[:],
                        op0=ALU.mult, op1=ALU.add)
                    c16 = cands[:].bitcast(u16)
                    for q in range(4):
                        nc.gpsimd.local_scatter(
                            hrec[:, bt, 512 * q:512 * (q + 1)].bitcast(u16),
                            c16[:, 128 * q:128 * (q + 1)],
                            idxe[:, 128 * q:128 * (q + 1)],
                            channels=128, num_elems=1024, num_idxs=128)

                    mv = pa.tile([128, 8 * MERGE_ROUNDS], f32,
                                 tag=f"mv{bt}", name=f"mv{bt}")
                    for r in range(MERGE_ROUNDS):
                        v8 = mv[:, 8 * r:8 * (r + 1)]
                        nc.vector.max(v8, merged[:])
                        if r < MERGE_ROUNDS - 1:
                            nc.vector.match_replace(merged[:], v8, merged[:], NEG)
                    thr = mv[:, KWIN - 1:KWIN]      # rank-102 value
                    if "nomerge" in ABL:
                        thr = mv[:, 0:1]
                    for q in range(4):
                        hq = hrec[:, bt, 512 * q:512 * (q + 1)]
                        nc.vector.scalar_tensor_tensor(
                            hq, hq, thr, hq, op0=ALU.is_ge, op1=ALU.mult)
                    if DBG and L == 1:
                        dd = nc.dram_tensor(f"dbg_thr{bt}", [128, 1], f32,
                                            kind="ExternalOutput")
                        nc.sync.dma_start(dd[:], thr)

                # --- I: transpose reconstructed rows into [u, k, b] layout ---
                nxt = pin.tile([128, KH, B], f32, tag="xin")
                tpa = pp_m.tile([128, 4, 128], f32, tag="psm", name="tpa")
                tpb = pp_y.tile([128, 4, 128], f32, tag="yp", name="tpb")
                tpd = [tpa, tpb]
                for c4 in range(KH // 4):
                    for bt in range(2):
                        buf = tpd[c4 % 2]
                        for ci in range(4):
                            c = 4 * c4 + ci
                            nc.tensor.transpose(
                                buf[:, ci, :],
                                hrec[:, bt, 128 * c:128 * (c + 1)], ident[:])
                        nc.scalar.copy(
                            nxt[:, 4 * c4:4 * c4 + 4, 128 * bt:128 * (bt + 1)],
                            buf[:])
                if DBG and L == 1:
                    dd = nc.dram_tensor("dbg_nxt", [128, KH, B], f32,
                                        kind="ExternalOutput")
                    nc.sync.dma_start(dd[:], nxt[:])
                return nxt

            def emit_head(h2T):
                wex = pa.tile([128, KH, OUT], f32, tag="wex")
                nc.sync.dma_start(
                    wex[:], dram["wexT"][:].rearrange("(k p) o -> p k o", p=128))
                wix = pa.tile([128, KH, 1], f32, tag="wix")
                nc.sync.dma_start(
                    wix[:], dram["wixT"][:].rearrange("(k p) o -> p k o", p=128))
                wei = pa.tile([1, OUT], f32, tag="wei")
                nc.sync.dma_start(wei[:], dram["weiT"][:])
                bo = pa.tile([1, OUT], f32, tag="bout")
                nc.sync.dma_start(bo[:], dram["bout"][:])

                # four accumulation chains (hwix, out) x (bt0, bt1)
                # interleaved per k-chunk so they consume transposed chunks
                # as the reconstruction pipeline produces them
                # PSUM bank discipline: a matmul's start=True clears the
                # whole bank, so each of the four interleaved chains gets its
                # own 2KB bank of the (dead) dendrite PSUM tiles
                hpd = pp_d.tile([128, NSEG, 128], f32, tag="d1", name="hpd")
                opd = pp_d.tile([128, NSEG, 128], f32, tag="d0", name="opd")
                hxs = [hpd[:, 0, 0:1], hpd[:, 4, 0:1]]
                tps = [hpd[0:1, 8, :], hpd[0:1, 9, :]]
                ops = [opd[:, 0, 0:OUT], opd[:, 4, 0:OUT]]
                for k in range(KH):
                    for bt in range(2):
                        sl = slice(128 * bt, 128 * (bt + 1))
                        nc.tensor.matmul(
                            hxs[bt], lhsT=h2T[:, k, sl],
                            rhs=wix[:, k, :], start=(k == 0), stop=(k == KH - 1))
                        nc.tensor.matmul(
                            ops[bt], lhsT=h2T[:, k, sl],
                            rhs=wex[:, k, :], start=(k == 0), stop=False)
                for bt in range(2):
                    # out -= (h2 @ Wix.T) @ Wei.T via negate + rank-1 update
                    nhx = pa.tile([128, 1], f32, tag=f"nhx{bt}",
                                  name=f"nhx{bt}")
                    nc.scalar.mul(nhx[:], hxs[bt], -1.0)
                    tp = tps[bt]
                    nc.tensor.transpose(tp, nhx[:], ident[:])
                    nhx_row = pa.tile([1, 128], f32, tag=f"nhxr{bt}",
                                      name=f"nhxr{bt}")
                    nc.scalar.copy(nhx_row[:], tp)
                    nc.tensor.matmul(ops[bt], lhsT=nhx_row[:], rhs=wei[:],
                                     start=False, stop=False)
                    nc.tensor.matmul(ops[bt], lhsT=ones[:], rhs=bo[:],
                                     start=False, stop=True)
                    ob = pa.tile([128, OUT], f32, tag=f"ob{bt}",
                                 name=f"ob{bt}")
                    nc.scalar.copy(ob[:], ops[bt])
                    nc.sync.dma_start(out_d[128 * bt:128 * (bt + 1)], ob[:])

            for it in range(n_iters):
                xT = pin.tile([128, KI, B], f32, tag="xin")
                # ACT-queue issue: keeps xT out of the SP queue ahead of the
                # latency-critical seg streams (ff needs it much later)
                nc.scalar.dma_start(
                    xT[:], dram["xT"][:].rearrange("(k p) b -> p k b", p=128))
                h1T = emit_layer(1, it, xT, KI)
                h2T = emit_layer(2, it, h1T, KH)
                emit_head(h2T)

    nc.compile()
    return nc


def _prep_inputs(inputs):
    """Host-side layout-only prep: shard + transpose + mask dtype cast."""
    import ml_dtypes
    np32 = lambda a: np.ascontiguousarray(np.asarray(a, dtype=np.float32))
    f8 = ml_dtypes.float8_e4m3
    common = {
        "xT": np.ascontiguousarray(np32(inputs["x"]).T),
        "cT": np.ascontiguousarray(np32(inputs["context"]).T),
        "wexT": np.ascontiguousarray(np32(inputs["Wex_out"]).T),
        "wixT": np.ascontiguousarray(np32(inputs["Wix_out"]).T),
        "weiT": np.ascontiguousarray(np32(inputs["Wei_out"]).T),
        "bout": np32(inputs["b_out"]).reshape(1, OUT),
    }
    in_maps = []
    for r in range(R):
        sh = slice(r * U, (r + 1) * U)
        m = dict(common)
        for L, (Wn, bn, sgn, mwn, msn) in {
            1: ("W1", "b1", "segW1", "maskW1", "maskS1"),
            2: ("W2", "b2", "segW2", "maskW2", "maskS2"),
        }.items():
            W = np32(inputs[Wn])[sh]          # [256, nin]
            mW = np32(inputs[mwn])[sh]
            sg = np32(inputs[sgn])[sh]        # [256, 10, 1024]
            msk = np32(inputs[msn])[sh]

            def seg_layout(a):
                # [u=256, s=10, c=1024] -> [c, uh=2, s, u128]
                t = a.transpose(2, 1, 0)                    # [c, s, u]
                t = t.reshape(D_CTX, NSEG, 2, 128)          # [c, s, uh, u]
                return np.ascontiguousarray(t.transpose(0, 2, 1, 3))

            m[f"wT{L}"] = np.ascontiguousarray(W.T)
            m[f"mwT{L}"] = np.ascontiguousarray(mW.T).astype(f8)
            m[f"sgT{L}"] = seg_layout(sg)
            m[f"msT{L}"] = seg_layout(msk).astype(f8)
            m[f"b{L}"] = np32(inputs[bn])[sh].reshape(1, U)
        in_maps.append(m)
    return in_maps


def kernel(**inputs) -> np.ndarray:
    global LAST_RESULT
    if "nc" not in _CACHE:
        _CACHE["nc"] = _build()
    in_maps = _prep_inputs(inputs)
    res = run_bass_kernel_spmd(_CACHE["nc"], in_maps, core_ids=list(range(R)))
    LAST_RESULT = res
    return np.asarray(res.results[0]["out"], dtype=np.float32)



# revision 8
# speedup vs baseline: 1.1995x; 1.1995x over previous
"""Trainium2 Bass kernel for nn_DaleDendriticMLP (topk_masking).

Tensor-parallel over the 2048 hidden units across 8 NeuronCores (256
units per core). K-winners and abs-argmax boundary gaps on this problem
are ~4e-7 relative, so all value-bearing matmuls must be fp32-exact-ish:
they are computed as 3-pass fp16 limb products (a = ah + al with fp16
limbs capturing 22 bits; d = ah@bh + ah@bl + al@bh, products exact in
fp32 PSUM, residual ~2^-23) which measures ~2.5e-7 end-to-end like the
HW fp32 mode but streams at 1 cycle/col instead of fp32's 4 (2 HW
passes x 2 cycles/col), and FWL (fp16 weight loads) hides LDWEIGHTS.

Weights (W*maskW, segW*maskS) are premasked and limb-split on the host
(weight preprocessing; no data-dependent host compute). x and context
are limb-split on device. Dendrite d[b,u,s] accumulates 8 context
chunks x 3 limb passes into PSUM; per-unit max/min over segments feed
the abs-argmax sigmoid gate (max on DVE, min on GPSIMD).

Per layer each core extracts its local top-32 gated values + unit
indices per sample and AllGathers them per batch-half (two 24KB
collectives per layer, launched as soon as that half's top-k is done).
Every core finds the exact per-row 102nd-largest of the merged 256
candidates (13 rounds of max8/match_replace), reconstructs the dense
hidden vector by gpsimd local_scatter of fp32 values as u16 lo/hi
pairs, applies the threshold mask, and PE-transposes into [unit,batch]
layout for the next layer.

Schedule: the L2 dendrite matmuls (the largest PE block) are emitted
between L1's top-k and L1's merge so the PE crunches them while the L1
AllGather + merge + scatter are in flight; L1's transposes and the ff2
matmuls follow. The Dale head streams wex with wix fused as column 100
(one matmul per chunk), and the rank-1 Wei@Wix correction is applied by
a single DVE op instead of PE transpose+matmuls.
"""

import os

os.environ.pop("JAX_PLATFORMS", None)
if not os.environ.get("BASS_TRACE"):
    os.environ["BASS_NEVER_TRACE"] = "1"

import numpy as np

import concourse.bacc as bacc
import concourse.tile as tile
import concourse.mybir as mybir
from concourse.bass_utils import run_bass_kernel_spmd

R = 8                    # cores
B = 256                  # batch
HID = 2048
U = HID // R             # 256 units per core
D_IN = 2048
D_CTX = 1024
KI = D_IN // 128         # 16 input K-chunks
KC = D_CTX // 128        # 8 context K-chunks
KH = HID // 128          # 16 hidden K-chunks
NSEG = 10
OUT = 100
KWIN = 102
LOC_ROUNDS = 4           # local top-32 per core
MERGE_ROUNDS = 13        # top-104 of merged 256
NEG = -1.0e30
VAL_N = 8 * LOC_ROUNDS   # 32 candidates per row per core per bt
# per-bt payload: 128 rows x 32 vals (f32) + 128 x 32 idx (u16 in f32 slots)
PAYB = 128 * VAL_N + 128 * VAL_N // 2    # 4096 + 2048 = 6144 f32

f32 = mybir.dt.float32
f16 = mybir.dt.float16
u16 = mybir.dt.uint16
i16 = mybir.dt.int16
X = mybir.AxisListType.X
ALU = mybir.AluOpType
AF = mybir.ActivationFunctionType

_CACHE = {}
LAST_RESULT = None
ABL = set(x for x in os.environ.get("ABL", "").split(",") if x)


def _build():
    nc = bacc.Bacc(
        "TRN2",
        target_bir_lowering=False,
        debug=False,
        enable_asserts=False,
        num_devices=R,
    )

    dram = {}

    def din(name, shape, dt=f32):
        dram[name] = nc.dram_tensor(name, shape, dt, kind="ExternalInput")
        return dram[name]

    din("xT", [D_IN, B])
    din("cT", [D_CTX, B])
    for L in (1, 2):
        din(f"sgh{L}", [D_CTX, 2, NSEG, 128], f16)
        din(f"sgl{L}", [D_CTX, 2, NSEG, 128], f16)
        din(f"wh{L}", [D_IN if L == 1 else HID, U], f16)
        din(f"wl{L}", [D_IN if L == 1 else HID, U], f16)
        din(f"b{L}", [1, U])
    din("wexT", [HID, OUT])
    din("wixT", [HID, 1])
    din("weiT", [1, OUT])
    din("bout", [1, OUT])
    out_d = nc.dram_tensor("out", [B, OUT], f32, kind="ExternalOutput")

    ident_d = nc.inline_tensor(np.eye(128, dtype=np.float32), "ident")
    ones_d = nc.inline_tensor(np.ones((1, 128), np.float32), "ones_row")
    # u16-scatter offsets: candidate block r (32 cands) lands in quarter r//2
    # at u16 base 512*(r%2); even/odd slots hold lo/hi halves
    j = np.arange(R * VAL_N)
    roffE_np = np.broadcast_to(
        (512 * ((j // VAL_N) % 2)).astype(np.uint16), (128, R * VAL_N))
    roffE_d = nc.inline_tensor(np.ascontiguousarray(roffE_np), "roffE")
    roffO_d = nc.inline_tensor(np.ascontiguousarray(roffE_np + 1), "roffO")

    gath_g = {
        (L, bt): nc.dram_tensor(f"gath_g{L}_{bt}", [R * PAYB], f32,
                                kind="Internal", addr_space="Shared")
        for L in (1, 2) for bt in range(2)
    }
    groups = [list(range(R))]

    with tile.TileContext(nc) as tc:
        with (
            tc.tile_pool(name="pa", bufs=1) as pa,          # persistent SBUF
            tc.tile_pool(name="pin", bufs=2) as pin,        # xT / nxt (f32)
            tc.tile_pool(name="plb", bufs=2) as plb,        # ff lhs limbs f16
            tc.tile_pool(name="pw", bufs=1) as pw,          # ff W limbs f16
            tc.tile_pool(name="psg", bufs=3) as psg,        # seg limb slices
            tc.tile_pool(name="pdram", bufs=2, space="DRAM") as pdram,
            tc.tile_pool(name="pp_d", bufs=1, space="PSUM") as pp_d,
            tc.tile_pool(name="pp_y", bufs=1, space="PSUM") as pp_y,
            tc.tile_pool(name="pp_m", bufs=1, space="PSUM") as pp_m,
        ):
            from concourse import library_config

            nc.gpsimd.load_library(library_config.local_scatter)

            ident = pa.tile([128, 128], f32, tag="ident")
            nc.sync.dma_start(ident[:], ident_d[:])
            ones = pa.tile([1, 128], f32, tag="ones")
            nc.sync.dma_start(ones[:], ones_d[:])
            roffE = pa.tile([128, R * VAL_N], u16, tag="roffE")
            nc.sync.dma_start(roffE[:], roffE_d[:])
            roffO = pa.tile([128, R * VAL_N], u16, tag="roffO")
            nc.sync.dma_start(roffO[:], roffO_d[:])

            cT = pa.tile([128, KC, B], f32, tag="cT")
            nc.sync.dma_start(
                cT[:], dram["cT"][:].rearrange("(k p) b -> p k b", p=128))

            # PE warmup: ~3.5us of dummy matmuls so HAM reaches K=8/8
            # before the first dendrite matmul issues
            warm = pa.tile([128, 512], f16, tag="warm")
            nc.vector.memset(warm[:], 0.0)
            wps = pp_m.tile([128, 4, 128], f32, tag="psm", name="wps")
            wfl = wps[:].rearrange("p a b -> p (a b)")
            for _ in range(10):
                nc.tensor.matmul(wfl, lhsT=warm[:, 0:128], rhs=warm[:],
                                 start=True, stop=True)

            # context limbs (device): ch + cl = cT to 22 bits
            ch = pa.tile([128, KC, B], f16, tag="ch")
            nc.vector.tensor_copy(ch[:], cT[:])
            chf = pa.tile([128, KC, B], f32, tag="hback", name="chf")
            nc.scalar.copy(chf[:], ch[:])
            cl = pa.tile([128, KC, B], f16, tag="cl")
            nc.vector.tensor_tensor(cl[:], cT[:], chf[:], op=ALU.subtract)

            # x limbs (device); xT streams on the ACT queue (needed at ff1)
            xT = pin.tile([128, KI, B], f32, tag="xin")
            nc.scalar.dma_start(
                xT[:], dram["xT"][:].rearrange("(k p) b -> p k b", p=128))
            xh = plb.tile([128, KI, B], f16, tag="fh")
            nc.vector.tensor_copy(xh[:], xT[:])
            xhf = pa.tile([128, KI, B], f32, tag="hback")
            nc.scalar.copy(xhf[:], xh[:])
            xl = plb.tile([128, KI, B], f16, tag="fl")
            nc.vector.tensor_tensor(xl[:], xT[:], xhf[:], op=ALU.subtract)

            def emit_seg_dma(L, uh, g2, eng):
                rows = slice(256 * g2, 256 * (g2 + 1))
                sgh_t = psg.tile([128, 2, NSEG * 128], f16, tag="sgh")
                eng.dma_start(
                    sgh_t[:].rearrange("p k (s u) -> p k s u", s=NSEG),
                    dram[f"sgh{L}"][rows, uh].rearrange(
                        "(k p) s u -> p k s u", p=128))
                sgl_t = psg.tile([128, 2, NSEG * 128], f16, tag="sgl")
                eng.dma_start(
                    sgl_t[:].rearrange("p k (s u) -> p k s u", s=NSEG),
                    dram[f"sgl{L}"][rows, uh].rearrange(
                        "(k p) s u -> p k s u", p=128))
                return sgh_t, sgl_t

            def emit_dend_mm(L, uh):
                """Dendrite matmuls for one 128-unit half; returns psums."""
                eng0 = nc.gpsimd if (L == 1 and uh == 0) else nc.sync
                sgs = [emit_seg_dma(L, uh, g2, eng0 if g2 == 0 else nc.sync)
                       for g2 in range(4)]
                dps = [pp_d.tile([128, NSEG, 128], f32, tag=f"d{bt}",
                                 name=f"d{L}{uh}{bt}") for bt in range(2)]
                dfl = [dps[bt][:].rearrange("p s u -> p (s u)")
                       for bt in range(2)]
                for g2 in range(4):
                    sgh_t, sgl_t = sgs[g2]
                    for k2 in range(2):
                        k = 2 * g2 + k2
                        for bt in range(2):
                            bsl = slice(128 * bt, 128 * (bt + 1))
                            for c0, ncols in ((0, 512), (512, 512),
                                              (1024, 256)):
                                dst = dfl[bt][:, c0:c0 + ncols]
                                nc.tensor.matmul(
                                    dst, lhsT=ch[:, k, bsl],
                                    rhs=sgh_t[:, k2, c0:c0 + ncols],
                                    start=(k == 0), stop=False)
                                nc.tensor.matmul(
                                    dst, lhsT=ch[:, k, bsl],
                                    rhs=sgl_t[:, k2, c0:c0 + ncols],
                                    start=False, stop=False)
                                nc.tensor.matmul(
                                    dst, lhsT=cl[:, k, bsl],
                                    rhs=sgh_t[:, k2, c0:c0 + ncols],
                                    start=False,
                                    stop=(k == KC - 1))
                return dps

            def emit_dend_red(dps, uh, maxd, mind):
                """Per-unit max/min over segments (DVE strided reduce)."""
                for bt in range(2):
                    v = dps[bt][:].rearrange("p s u -> p u s")
                    col = U * bt + 128 * uh
                    nc.vector.tensor_reduce(
                        maxd[:, col:col + 128], v, axis=X, op=ALU.max)
                    nc.vector.tensor_reduce(
                        mind[:, col:col + 128], v, axis=X, op=ALU.min)

            def emit_ff(L, fh, fl, nk, y_all):
                wh_t = pw.tile([128, nk, U], f16, tag="wh", name=f"wh{L}")
                nc.sync.dma_start(
                    wh_t[:],
                    dram[f"wh{L}"][:].rearrange("(k p) u -> p k u", p=128))
                wl_t = pw.tile([128, nk, U], f16, tag="wl", name=f"wl{L}")
                nc.sync.dma_start(
                    wl_t[:],
                    dram[f"wl{L}"][:].rearrange("(k p) u -> p k u", p=128))
                b_sb = pa.tile([1, U], f32, tag="bias", name=f"bias{L}")
                nc.scalar.dma_start(b_sb[:], dram[f"b{L}"][:])
                for bt in range(2):
                    bsl = slice(128 * bt, 128 * (bt + 1))
                    yp = pp_y.tile([128, 2, U], f32, tag="yp",
                                   name=f"yp{L}{bt}")
                    for k in range(nk):
                        nc.tensor.matmul(yp[:, 0, :], lhsT=fh[:, k, bsl],
                                         rhs=wh_t[:, k, :],
                                         start=(k == 0), stop=False)
                        nc.tensor.matmul(yp[:, 0, :], lhsT=fh[:, k, bsl],
                                         rhs=wl_t[:, k, :],
                                         start=False, stop=False)
                        nc.tensor.matmul(yp[:, 0, :], lhsT=fl[:, k, bsl],
                                         rhs=wh_t[:, k, :],
                                         start=False, stop=False)
                    nc.tensor.matmul(yp[:, 0, :], lhsT=ones[:], rhs=b_sb[:],
                                     start=False, stop=True)
                    nc.scalar.copy(y_all[:, U * bt:U * (bt + 1)], yp[:, 0, :])

            def emit_gate(L, maxd, mind, y_all, yg):
                g = pa.tile([128, 2 * U], f32, tag="g")
                nc.vector.tensor_tensor(g[:], maxd[:], mind[:], op=ALU.add)
                ga = pa.tile([128, 2 * U], f32, tag="ga")
                nc.vector.scalar_tensor_tensor(
                    ga[:], g[:], 0.0, maxd[:], op0=ALU.is_ge, op1=ALU.mult)
                gb = pa.tile([128, 2 * U], f32, tag="gb")
                nc.vector.scalar_tensor_tensor(
                    gb[:], g[:], 0.0, mind[:], op0=ALU.is_lt, op1=ALU.mult)
                nc.vector.tensor_tensor(ga[:], ga[:], gb[:], op=ALU.add)
                sig = pa.tile([128, 2 * U], f32, tag="sig")
                nc.scalar.activation(sig[:], ga[:], AF.Sigmoid)
                nc.vector.tensor_tensor(yg[:], y_all[:], sig[:], op=ALU.mult)

            def emit_topk_ag(L, bt, yg):
                """Local top-32 of this bt half, payload DMA + AllGather."""
                vals_c = pa.tile([128, VAL_N], f32, tag=f"vals{bt}")
                idx_c = pa.tile([128, VAL_N], u16, tag=f"idxc{bt}")
                sc = yg[:, U * bt:U * (bt + 1)]
                for r in range(LOC_ROUNDS):
                    v8 = vals_c[:, 8 * r:8 * (r + 1)]
                    nc.vector.max(v8, sc)
                    nc.vector.max_index(idx_c[:, 8 * r:8 * (r + 1)], v8, sc)
                    if r < LOC_ROUNDS - 1:
                        nc.vector.match_replace(sc, v8, sc, NEG)
                pay = pdram.tile([PAYB], f32, tag="pay")
                nc.gpsimd.dma_start(
                    pay[0:128 * VAL_N].rearrange("(p j) -> p j", p=128),
                    vals_c[:])
                nc.gpsimd.dma_start(
                    pay[128 * VAL_N:].bitcast(u16).rearrange(
                        "(p j) -> p j", p=128),
                    idx_c[:])
                if "nocc" in ABL:
                    nc.gpsimd.dma_start(gath_g[(L, bt)][0:PAYB], pay[:])
                else:
                    nc.gpsimd.collective_compute(
                        "AllGather", ALU.bypass, replica_groups=groups,
                        ins=[pay.opt()], outs=[gath_g[(L, bt)][:]])

            def emit_gather_idxe(L, bt, hrec):
                """Post-AG: pull candidates into SBUF, expand scatter idx,
                scatter raw values (no threshold dep)."""
                gath = gath_g[(L, bt)]
                merged = pa.tile([128, R * VAL_N], f32, tag=f"mrg{bt}",
                                 name=f"mrg{L}{bt}")
                nc.gpsimd.dma_start(
                    merged[:].rearrange("p (r j) -> p r j", r=R),
                    gath[:].rearrange("(r q) -> r q", q=PAYB)
                    [:, 0:128 * VAL_N].rearrange("r (p j) -> p r j", p=128))
                cands = pa.tile([128, R * VAL_N], f32, tag=f"cnd{bt}",
                                name=f"cnd{L}{bt}")
                nc.vector.tensor_copy(cands[:], merged[:])
                idxg = pa.tile([128, R * VAL_N], u16, tag=f"idg{bt}",
                               name=f"idg{L}{bt}")
                nc.gpsimd.dma_start(
                    idxg[:].rearrange("p (r j) -> p r j", r=R),
                    gath[:].rearrange("(r q) -> r q", q=PAYB)
                    [:, 128 * VAL_N:].bitcast(u16).rearrange(
                        "r (p j) -> p r j", p=128))
                idxe = pa.tile([128, 2 * R * VAL_N], i16, tag=f"ide{bt}",
                               name=f"ide{L}{bt}")
                iev = idxe[:].rearrange("p (j t) -> p t j", t=2)
                nc.vector.scalar_tensor_tensor(
                    iev[:, 0, :], idxg[:], 2.0, roffE[:],
                    op0=ALU.mult, op1=ALU.add)
                nc.vector.scalar_tensor_tensor(
                    iev[:, 1, :], idxg[:], 2.0, roffO[:],
                    op0=ALU.mult, op1=ALU.add)
                c16 = cands[:].bitcast(u16)
                for q in range(4):
                    nc.gpsimd.local_scatter(
                        hrec[:, bt, 512 * q:512 * (q + 1)].bitcast(u16),
                        c16[:, 128 * q:128 * (q + 1)],
                        idxe[:, 128 * q:128 * (q + 1)],
                        channels=128, num_elems=1024, num_idxs=128)
                return merged

            def emit_merge_thresh(L, bt, merged, hrec):
                mv = pa.tile([128, 8 * MERGE_ROUNDS], f32, tag=f"mv{bt}",
                             name=f"mv{L}{bt}")
                for r in range(MERGE_ROUNDS):
                    v8 = mv[:, 8 * r:8 * (r + 1)]
                    nc.vector.max(v8, merged[:])
                    if r < MERGE_ROUNDS - 1:
                        nc.vector.match_replace(merged[:], v8, merged[:], NEG)
                thr = mv[:, KWIN - 1:KWIN]
                for q in range(4):
                    hq = hrec[:, bt, 512 * q:512 * (q + 1)]
                    nc.vector.scalar_tensor_tensor(
                        hq, hq, thr, hq, op0=ALU.is_ge, op1=ALU.mult)

            def emit_trans(L, hrec, nxt, c4):
                """Transpose one 4-chunk group of both bt halves into nxt."""
                if c4 % 2 == 0:
                    buf = pp_m.tile([128, 4, 128], f32, tag="psm",
                                    name=f"tp{L}{c4}")
                else:
                    b2 = pp_y.tile([128, 2, U], f32, tag="yp",
                                   name=f"tp{L}{c4}")
                    buf = b2[:].rearrange("p a (c d) -> p (a c) d", d=128)
                for bt in range(2):
                    for ci in range(4):
                        c = 4 * c4 + ci
                        nc.tensor.transpose(
                            buf[:, ci, :],
                            hrec[:, bt, 128 * c:128 * (c + 1)], ident[:])
                    nc.scalar.copy(
                        nxt[:, 4 * c4:4 * c4 + 4, 128 * bt:128 * (bt + 1)],
                        buf[:])

            def emit_limb_group(nxt, hh, hl, c4):
                sl = slice(4 * c4, 4 * c4 + 4)
                nc.vector.tensor_copy(hh[:, sl, :], nxt[:, sl, :])
                hbf = pa.tile([128, 4, B], f32, tag="hback",
                              name=f"hback{c4}")
                nc.scalar.copy(hbf[:], hh[:, sl, :])
                nc.vector.tensor_tensor(hl[:, sl, :], nxt[:, sl, :], hbf[:],
                                        op=ALU.subtract)

            # ================= schedule =================
            # --- L1 dendrites + ff + gate + topk ---
            maxd1 = pa.tile([128, 2 * U], f32, tag="maxd")
            mind1 = pa.tile([128, 2 * U], f32, tag="mind")
            for uh in range(2):
                dps = emit_dend_mm(1, uh)
                emit_dend_red(dps, uh, maxd1, mind1)

            y1 = pa.tile([128, 2 * U], f32, tag="y_all")
            emit_ff(1, xh, xl, KI, y1)
            yg1 = pa.tile([128, 2 * U], f32, tag="yg")
            emit_gate(1, maxd1, mind1, y1, yg1)
            emit_topk_ag(1, 0, yg1)
            emit_topk_ag(1, 1, yg1)

            # --- L2 dendrite matmuls run on the PE while AG1 + merge +
            # scatter are in flight; reduces are emitted so they never
            # block the L1 post-AG chain in the DVE/GPSIMD queues ---
            maxd2 = pa.tile([128, 2 * U], f32, tag="maxd2")
            mind2 = pa.tile([128, 2 * U], f32, tag="mind2")
            dps20 = emit_dend_mm(2, 0)
            dps21 = emit_dend_mm(2, 1)

            hrec1 = pa.tile([128, 2, HID], f32, tag="hrec", name="hrec1")
            mrg = [emit_gather_idxe(1, bt, hrec1) for bt in range(2)]
            emit_dend_red(dps20, 0, maxd2, mind2)
            for bt in range(2):
                emit_merge_thresh(1, bt, mrg[bt], hrec1)
            emit_dend_red(dps21, 1, maxd2, mind2)

            # head weights (needed only at the tail; loaded mid-kernel so
            # they never compete with the seg streams)
            wexf = pa.tile([128, KH, OUT + 1], f32, tag="wexf")
            nc.sync.dma_start(
                wexf[:, :, 0:OUT],
                dram["wexT"][:].rearrange("(k p) o -> p k o", p=128))
            nc.sync.dma_start(
                wexf[:, :, OUT:OUT + 1],
                dram["wixT"][:].rearrange("(k p) o -> p k o", p=128))
            wei = pa.tile([1, OUT], f32, tag="wei")
            nc.sync.dma_start(wei[:], dram["weiT"][:])
            boutx = pa.tile([1, OUT + 1], f32, tag="boutx")
            nc.vector.memset(boutx[:], 0.0)
            nc.sync.dma_start(boutx[:, 0:OUT], dram["bout"][:])

            nxt = pin.tile([128, KH, B], f32, tag="xin", name="h1T")
            hh = plb.tile([128, KH, B], f16, tag="fh", name="h1h")
            hl = plb.tile([128, KH, B], f16, tag="fl", name="h1l")
            for c4 in range(4):
                emit_trans(1, hrec1, nxt, c4)
                emit_limb_group(nxt, hh, hl, c4)

            # wei broadcast to [128, OUT] via PE for the DVE rank-1 fixup
            wps2 = pp_m.tile([128, 4, 128], f32, tag="psm", name="wps2")
            nc.tensor.matmul(wps2[:, 0, 0:OUT], lhsT=ones[:], rhs=wei[:],
                             start=True, stop=True)
            wei128 = pa.tile([128, OUT], f32, tag="wei128")
            nc.scalar.copy(wei128[:], wps2[:, 0, 0:OUT])

            y2 = pa.tile([128, 2 * U], f32, tag="y_all", name="y2")
            emit_ff(2, hh, hl, KH, y2)
            yg2 = pa.tile([128, 2 * U], f32, tag="yg", name="yg2")
            emit_gate(2, maxd2, mind2, y2, yg2)
            emit_topk_ag(2, 0, yg2)
            emit_topk_ag(2, 1, yg2)

            hrec2 = pa.tile([128, 2, HID], f32, tag="hrec", name="hrec2")
            mrg2 = [emit_gather_idxe(2, bt, hrec2) for bt in range(2)]
            for bt in range(2):
                emit_merge_thresh(2, bt, mrg2[bt], hrec2)

            # tail: transposes interleaved with head accumulation chains
            h2T = pin.tile([128, KH, B], f32, tag="xin", name="h2T")
            hpd = pp_d.tile([128, NSEG, 128], f32, tag="d0", name="head_ps")
            hps = [hpd[:, 0, 0:OUT + 1], hpd[:, 4, 0:OUT + 1]]
            for c4 in range(4):
                emit_trans(2, hrec2, h2T, c4)
                for k in range(4 * c4, 4 * c4 + 4):
                    for bt in range(2):
                        bsl = slice(128 * bt, 128 * (bt + 1))
                        nc.tensor.matmul(
                            hps[bt], lhsT=h2T[:, k, bsl],
                            rhs=wexf[:, k, :], start=(k == 0), stop=False)
            for bt in range(2):
                nc.tensor.matmul(hps[bt], lhsT=ones[:], rhs=boutx[:],
                                 start=False, stop=True)
                ob = pa.tile([128, OUT], f32, tag=f"ob{bt}")
                nc.scalar.copy(ob[:], hps[bt][:, 0:OUT])
                nhx = pa.tile([128, 1], f32, tag=f"nhx{bt}")
                nc.scalar.mul(nhx[:], hps[bt][:, OUT:OUT + 1], -1.0)
                nc.vector.scalar_tensor_tensor(
                    ob[:], wei128[:], nhx[:], ob[:],
                    op0=ALU.mult, op1=ALU.add)
                nc.sync.dma_start(out_d[128 * bt:128 * (bt + 1)], ob[:])

    nc.compile()
    return nc


def _prep_inputs(inputs):
    """Host prep: shard, transpose, premask weights, fp16 limb split."""
    np32 = lambda a: np.ascontiguousarray(np.asarray(a, dtype=np.float32))

    def limbs(a):
        h = a.astype(np.float16)
        l = (a - h.astype(np.float32)).astype(np.float16)
        return np.ascontiguousarray(h), np.ascontiguousarray(l)

    common = {
        "xT": np.ascontiguousarray(np32(inputs["x"]).T),
        "cT": np.ascontiguousarray(np32(inputs["context"]).T),
        "wexT": np.ascontiguousarray(np32(inputs["Wex_out"]).T),
        "wixT": np.ascontiguousarray(np32(inputs["Wix_out"]).T),
        "weiT": np.ascontiguousarray(np32(inputs["Wei_out"]).T),
        "bout": np32(inputs["b_out"]).reshape(1, OUT),
    }
    in_maps = []
    for r in range(R):
        sh = slice(r * U, (r + 1) * U)
        m = dict(common)
        for L, (Wn, bn, sgn, mwn, msn) in {
            1: ("W1", "b1", "segW1", "maskW1", "maskS1"),
            2: ("W2", "b2", "segW2", "maskW2", "maskS2"),
        }.items():
            Wm = np32(inputs[Wn])[sh] * np32(inputs[mwn])[sh]   # [256, nin]
            sgm = np32(inputs[sgn])[sh] * np32(inputs[msn])[sh]

            def seg_layout(a):
                # [u=256, s=10, c=1024] -> [c, uh=2, s, u128]
                t = a.transpose(2, 1, 0)                    # [c, s, u]
                t = t.reshape(D_CTX, NSEG, 2, 128)
                return np.ascontiguousarray(t.transpose(0, 2, 1, 3))

            sgh, sgl = limbs(seg_layout(sgm))
            wh, wl = limbs(np.ascontiguousarray(Wm.T))
            m[f"sgh{L}"] = sgh
            m[f"sgl{L}"] = sgl
            m[f"wh{L}"] = wh
            m[f"wl{L}"] = wl
            m[f"b{L}"] = np32(inputs[bn])[sh].reshape(1, U)
        in_maps.append(m)
    return in_maps


def kernel(**inputs) -> np.ndarray:
    global LAST_RESULT
    if "nc" not in _CACHE:
        _CACHE["nc"] = _build()
    in_maps = _prep_inputs(inputs)
    res = run_bass_kernel_spmd(_CACHE["nc"], in_maps, core_ids=list(range(R)))
    LAST_RESULT = res
    return np.asarray(res.results[0]["out"], dtype=np.float32)
